# revision 2
# baseline (speedup 1.0000x reference)
"""GroupedTernaryLinear Trainium2 kernel v2 (Bass/Tile, 8-core SPMD).

Computation (matches the jax reference):
  x:      [2, 4096, 4096] f32   -> flatten to [8192, 4096] tokens
  weight: [4096, 1024]    f32
  1. xn = rms_norm(x) over last dim (eps = f32 eps)
  2. w_bf = bf16(weight); per flat 64-chunk: scale = bf16(mean|w_bf|) (clipped),
     q = clip(round(w_bf/scale), -1, 1)  ->  wq = q*scale  (exact in bf16)
  3. out[t, g*1024+o] = sum_i xn[t, g*1024+i] * wq[g*1024+o, i]   (4 groups)

Sharding: 2 token-halves x 4 groups = 8 cores. Core c = 4*i + j gets
tokens [4096*i, 4096*(i+1)) and group j. The rms-norm needs the full
4096-feature sum of squares per token; the 4 group-shards of each token
half AllReduce their per-token partials in two 16-block chunks; the norm
factor is folded into the output evacuation.

v2 design (vs the 219us two-pass baseline):
  - The PE runs ONLY the 512 main matmuls. All transposes (x and wq) go
    through the DMA XBAR (dma_start transpose=True, bf16) straight into
    the [128, k, .] layouts the matmuls read. This removes 40960 PE rows
    and keeps the tensor queue dense so the HAM clock gate stays warm.
  - Single interleaved pass: each block streams lo+hi (1024 cols) per
    stationary load and its full [128, 1024] output is evacuated and
    written out immediately -> output DMA is uniform across the kernel
    instead of piling into a 25us tail.
  - Blocks whose norm factor hasn't arrived stage raw bf16 outputs and
    are flushed 2-per-iteration once the factor lands; late blocks scale
    straight out of PSUM.
  - First 6 blocks run on the lo half only (h0-split) so the PE starts
    ~4us earlier, while tiles 4-7 still quantize.
  - Queue plan keeps fac-waiting ops off the engines that feed the PE:
    sync: x-in DMA + x transposes (+1/3 of out DMA)
    scalar: w DMA + w-bf cast + w transposes + squares + flush-lo scale
    vector: w quant + PSUM evacuation (direct scale / bf16 staging)
    gpsimd: x bf16 cast + collectives (+1/3 of out DMA)
"""

import os
import sys

sys.path.insert(0, "/opt/trn_rl_repo")

import numpy as np

import concourse.bass as bass
import concourse.mybir as mybir
import concourse.tile as tile
from concourse import bacc
from concourse.bass_utils import run_bass_kernel_spmd

F32 = mybir.dt.float32
BF16 = mybir.dt.bfloat16
AF = mybir.ActivationFunctionType
ALU = mybir.AluOpType

N_CORES = 8
TOK = 4096        # tokens per core
DIN = 1024        # per-core input features (one group)
DOUT = 1024       # per-core outputs (one group)
DFULL = 4096      # full feature dim (norm denominator)
TB = TOK // 128   # 32 token blocks
GK = DIN // 128   # 8 k-chunks of 128
EPS = 1.1920929e-07          # np.finfo(np.float32).eps
THR = 0.5009765625           # bf16 round-to-nearest-even threshold for |r|>0.5

H0S = 6           # blocks run as split lo-then-hi while w tiles 4-7 quantize
DIRECT_FROM = 26  # blocks from here scale straight out of PSUM
CC_SPLIT = 16     # ss AllReduce chunk boundary
FLUSH_A_ITER = 13  # main-loop iter where fac_a chain + chunk-A flushes start
FLUSH_B_ITER = 26  # same for chunk B

LAST_EXEC_NS = None
LAST_RESULTS = None


def _build():
    nc = bacc.Bacc("TRN2", target_bir_lowering=False, debug=False, num_devices=8)
    x_ap = nc.dram_tensor("x", [TOK, DIN], F32, kind="ExternalInput").ap()
    w_ap = nc.dram_tensor("weight", [DOUT, DIN], F32, kind="ExternalInput").ap()
    out_ap = nc.dram_tensor("out", [TOK, DOUT], F32, kind="ExternalOutput").ap()

    with tile.TileContext(nc) as tc:
        _body(tc, nc, out_ap, x_ap, w_ap)

    nc.compile()
    return nc


def _body(tc, nc, out_ap, x_ap, w_ap):
    with (
        tc.tile_pool(name="consts", bufs=1) as consts,
        tc.tile_pool(name="wqt", bufs=1) as wqt_pool,
        tc.tile_pool(name="rawp", bufs=1) as raw_pool,
        tc.tile_pool(name="win", bufs=4) as win_pool,
        tc.tile_pool(name="wtmp", bufs=2) as wtmp_pool,
        tc.tile_pool(name="wst", bufs=2) as wst_pool,
        tc.tile_pool(name="xin", bufs=5) as xin_pool,
        tc.tile_pool(name="xbp", bufs=4) as xb_pool,
        tc.tile_pool(name="xta", bufs=14) as xta_pool,
        tc.tile_pool(name="stats", bufs=1) as stats_pool,
        tc.tile_pool(name="obp", bufs=8) as ob_pool,
        tc.tile_pool(name="dram", bufs=1, space="DRAM") as dram_pool,
        tc.tile_pool(name="ps_mm", bufs=6, space="PSUM") as ps_mm,
    ):
        eps_t = consts.tile([128, 1], F32, name="eps_t")
        nc.vector.memset(eps_t[:], EPS)

        # Quantized transposed weight: [i(128), k, o-half]
        wqT_lo = wqt_pool.tile([128, GK, 512], BF16, name="wqT_lo")
        wqT_hi = wqt_pool.tile([128, GK, 512], BF16, name="wqT_hi")

        ss_all = stats_pool.tile([128, TB], F32, name="ss_all")
        ss_sum_a = stats_pool.tile([128, CC_SPLIT], F32, name="ss_sum_a")
        ss_sum_b = stats_pool.tile([128, TB - CC_SPLIT], F32, name="ss_sum_b")
        sq_a = stats_pool.tile([128, CC_SPLIT], F32, name="sq_a")
        sq_b = stats_pool.tile([128, TB - CC_SPLIT], F32, name="sq_b")
        fac_a = stats_pool.tile([128, CC_SPLIT], F32, name="fac_a")
        fac_b = stats_pool.tile([128, TB - CC_SPLIT], F32, name="fac_b")
        junk = stats_pool.tile([128, DIN], BF16, name="junk")

        cc_in_a = dram_pool.tile([128, CC_SPLIT], F32, name="cc_in_a")
        cc_out_a = dram_pool.tile([128, CC_SPLIT], F32, name="cc_out_a")
        cc_in_b = dram_pool.tile([128, TB - CC_SPLIT], F32, name="cc_in_b")
        cc_out_b = dram_pool.tile([128, TB - CC_SPLIT], F32, name="cc_out_b")

        def fac_ap(b):
            if b < CC_SPLIT:
                return fac_a[:, b:b + 1]
            return fac_b[:, b - CC_SPLIT:b - CC_SPLIT + 1]

        w_tiles = {}
        x_pending = []   # (b, f32 tile) in DMA flight
        xT = {}          # b -> [128, GK, 128] bf16 transposed tile
        staged = {}      # b -> ("full", tile) | ("halves", lo, hi)
        pm_live = {}     # b -> (pm_lo, pm_hi) awaiting evacuation

        def emit_wdma(t):
            w_t = win_pool.tile([128, DIN], F32, name="w_t")
            nc.scalar.dma_start(w_t[:], w_ap[t * 128:(t + 1) * 128, :])
            w_tiles[t] = w_t

        def emit_wquant(t):
            w_t = w_tiles.pop(t)
            wbf = wtmp_pool.tile([128, DIN], BF16, name="wbf")
            nc.scalar.copy(wbf[:], w_t[:])              # f32 -> bf16 (RNE)
            wbf_v = wbf[:].rearrange("p (c q) -> p c q", q=64)

            red = wst_pool.tile([128, 16], F32, name="red")
            nc.vector.tensor_reduce(
                red[:], wbf_v, axis=mybir.AxisListType.X, op=ALU.add,
                apply_absolute_value=True,
            )
            s_bf = wst_pool.tile([128, 16], BF16, name="s_bf")
            nc.vector.tensor_scalar(
                s_bf[:], red[:], 1.0 / 64.0, 1e-8, ALU.mult, ALU.max,
            )
            thr_p = wst_pool.tile([128, 16], F32, name="thr_p")
            nc.vector.tensor_scalar_mul(thr_p[:], s_bf[:], THR)
            thr_n = wst_pool.tile([128, 16], F32, name="thr_n")
            nc.vector.tensor_scalar_mul(thr_n[:], s_bf[:], -THR)

            # q = (w > t) - (w < -t); wq = q*s  (broadcast views)
            tp_b = thr_p[:].unsqueeze(2).broadcast_to((128, 16, 64))
            tn_b = thr_n[:].unsqueeze(2).broadcast_to((128, 16, 64))
            s_b = s_bf[:].unsqueeze(2).broadcast_to((128, 16, 64))
            mp = wtmp_pool.tile([128, DIN], BF16, name="mp")
            mp_v = mp[:].rearrange("p (c q) -> p c q", q=64)
            nc.vector.tensor_tensor(mp_v, wbf_v, tp_b, ALU.is_gt)
            mn = wtmp_pool.tile([128, DIN], BF16, name="mn")
            mn_v = mn[:].rearrange("p (c q) -> p c q", q=64)
            nc.vector.tensor_tensor(mn_v, wbf_v, tn_b, ALU.is_lt)
            nc.vector.tensor_sub(mp[:], mp[:], mn[:])
            wqv = wtmp_pool.tile([128, DIN], BF16, name="wqv")
            wqv_v = wqv[:].rearrange("p (c q) -> p c q", q=64)
            nc.vector.tensor_tensor(wqv_v, mp_v, s_b, ALU.mult)

            # XBAR transpose into the resident weight tile:
            # wqT[i, k, (t%4)*128+o] = wqv[o, k*128+i]
            dst = wqT_lo if t < 4 else wqT_hi
            off = (t % 4) * 128
            nc.scalar.dma_start(
                dst[:, :, off:off + 128], wqv[:], transpose=True,
            )

        def emit_xdma(b):
            x_t = xin_pool.tile([128, DIN], F32, name="x_t")
            nc.sync.dma_start(x_t[:], x_ap[b * 128:(b + 1) * 128, :])
            x_pending.append((b, x_t))

        def emit_xchain(b):
            if b + 5 < TB:
                emit_xdma(b + 5)
            bb, x_t = x_pending.pop(0)
            assert bb == b, (bb, b)
            nc.scalar.activation(
                junk[:], x_t[:], AF.Square, accum_out=ss_all[:, b:b + 1],
            )
            xb = xb_pool.tile([128, DIN], BF16, name="xb")
            nc.gpsimd.tensor_copy(xb[:], x_t[:])
            xt = xta_pool.tile([128, GK, 128], BF16, name="xt")
            nc.sync.dma_start(xt[:], xb[:], transpose=True)
            xT[b] = xt

        def emit_mm8(b, half):
            # h0-split path: one 512-col half, staged to bf16 immediately.
            w = wqT_lo if half == 0 else wqT_hi
            pm = ps_mm.tile([128, 512], F32, name="pm")
            for k in range(GK):
                nc.tensor.matmul(
                    pm[:], xT[b][:, k, :], w[:, k, :],
                    start=(k == 0), stop=(k == GK - 1),
                )
            rhh = raw_pool.tile([128, 512], BF16, name=f"rh{b}_{half}")
            nc.vector.tensor_copy(rhh[:], pm[:])
            if half == 0:
                staged[b] = ["halves", rhh, None]
            else:
                staged[b][2] = rhh

        def emit_mm16(b):
            pml = ps_mm.tile([128, 512], F32, name="pm")
            pmh = ps_mm.tile([128, 512], F32, name="pm")
            for k in range(GK):
                nc.tensor.matmul(
                    pml[:], xT[b][:, k, :], wqT_lo[:, k, :],
                    start=(k == 0), stop=(k == GK - 1),
                )
                nc.tensor.matmul(
                    pmh[:], xT[b][:, k, :], wqT_hi[:, k, :],
                    start=(k == 0), stop=(k == GK - 1),
                )
            pm_live[b] = (pml, pmh)

        def out_dma(b, ob):
            oeng = (nc.gpsimd, nc.sync, nc.scalar)[b % 3]
            oeng.dma_start(out_ap[b * 128:(b + 1) * 128, :], ob[:])

        def emit_evac(b):
            pml, pmh = pm_live.pop(b)
            if b >= DIRECT_FROM:
                ob = ob_pool.tile([128, DOUT], F32, name="ob")
                nc.vector.tensor_scalar_mul(ob[:, 0:512], pml[:], fac_ap(b))
                nc.vector.tensor_scalar_mul(ob[:, 512:1024], pmh[:], fac_ap(b))
                out_dma(b, ob)
            else:
                rhb = raw_pool.tile([128, DOUT], BF16, name=f"rh{b}")
                nc.vector.tensor_copy(rhb[:, 0:512], pml[:])
                nc.vector.tensor_copy(rhb[:, 512:1024], pmh[:])
                staged[b] = ["full", rhb, None]

        def emit_flush(b):
            ent = staged.pop(b)
            ob = ob_pool.tile([128, DOUT], F32, name="ob")
            if ent[0] == "full":
                rhb = ent[1]
                lo, hi = rhb[:, 0:512], rhb[:, 512:1024]
            else:
                lo, hi = ent[1][:], ent[2][:]
            nc.scalar.activation(ob[:, 0:512], lo, AF.Copy, scale=fac_ap(b))
            nc.vector.tensor_scalar_mul(ob[:, 512:1024], hi, fac_ap(b))
            out_dma(b, ob)

        def emit_cc(chunk):
            if chunk == 0:
                cc_in, cc_out, sl = cc_in_a, cc_out_a, slice(0, CC_SPLIT)
            else:
                cc_in, cc_out, sl = cc_in_b, cc_out_b, slice(CC_SPLIT, TB)
            nc.gpsimd.dma_start(cc_in[:], ss_all[:, sl])
            nc.gpsimd.collective_compute(
                "AllReduce",
                ALU.add,
                replica_groups=[[0, 1, 2, 3], [4, 5, 6, 7]],
                ins=[cc_in.opt()],
                outs=[cc_out.opt()],
            )

        def emit_fac(chunk):
            if chunk == 0:
                cc_out, ss_sum, sq, fac = cc_out_a, ss_sum_a, sq_a, fac_a
            else:
                cc_out, ss_sum, sq, fac = cc_out_b, ss_sum_b, sq_b, fac_b
            nc.gpsimd.dma_start(ss_sum[:], cc_out[:])
            nc.scalar.activation(
                sq[:], ss_sum[:], AF.Sqrt, bias=eps_t[:], scale=1.0 / DFULL,
            )
            nc.vector.reciprocal(fac[:], sq[:])

        # ---- emission ------------------------------------------------------
        # Interleaved initial DMA issue: w-lo tiles + first x blocks share
        # bandwidth so both the weight and the x pipeline start promptly.
        emit_wdma(0)
        emit_wdma(1)
        emit_xdma(0)
        emit_xdma(1)
        emit_wdma(2)
        emit_wdma(3)
        for b in range(2, 5):
            emit_xdma(b)
        for t in range(4):
            emit_wquant(t)

        # h0 phase: blocks 0..H0S-1 on the lo half; w tiles 4-7 arrive and
        # quantize meanwhile.
        for b in range(H0S):
            emit_xchain(b)
            if b == 0:
                for t in range(4, 8):
                    emit_wdma(t)
            emit_mm8(b, 0)
            if b < 4:
                emit_wquant(4 + b)
        # h1 catch-up for the split blocks; keep the x pipeline moving.
        for b in range(H0S):
            emit_xchain(H0S + b)
            emit_mm8(b, 1)

        # main loop
        flushq = list(range(CC_SPLIT)) + list(range(CC_SPLIT, DIRECT_FROM))
        next_chain = 2 * H0S
        cc_a_done = cc_b_done = False
        MM_LAG = 1
        for b in range(H0S, TB):
            n_chains = 2 if b < H0S + 3 else 1
            for _ in range(n_chains):
                if next_chain < TB:
                    emit_xchain(next_chain)
                    next_chain += 1
                    if next_chain == CC_SPLIT:
                        emit_cc(0)
                        cc_a_done = True
                    elif next_chain == TB:
                        emit_cc(1)
                        cc_b_done = True
            emit_mm16(b)
            if b - MM_LAG in pm_live:
                emit_evac(b - MM_LAG)
            if b == FLUSH_A_ITER:
                emit_fac(0)
            if b == FLUSH_B_ITER:
                emit_fac(1)
            nflush = 0
            while nflush < 2 and flushq:
                fb = flushq[0]
                ok = (b >= FLUSH_A_ITER) if fb < CC_SPLIT else (b >= FLUSH_B_ITER)
                if not ok or fb not in staged:
                    break
                emit_flush(flushq.pop(0))
                nflush += 1
        # drain
        for b in sorted(pm_live):
            emit_evac(b)
        for fb in flushq:
            if fb in staged:
                emit_flush(fb)


_NC_CACHE = None


def _ensure_ntff_hook():
    """Install the antenv.axon_hooks shim + ctypes NTFF hook if missing."""
    import types

    try:
        from antenv.axon_hooks import get_axon_ntff_profile_hook  # noqa: F401
        return
    except ImportError:
        pass
    import antenv

    mod = types.ModuleType("antenv.axon_hooks")
    mod._hook = None
    mod.set_axon_ntff_profile_hook = lambda h: setattr(mod, "_hook", h)
    mod.get_axon_ntff_profile_hook = lambda: mod._hook
    sys.modules["antenv.axon_hooks"] = mod
    antenv.axon_hooks = mod
    try:
        if "/root/.axon_site" not in sys.path:
            sys.path.insert(0, "/root/.axon_site")
        from trn_agent_boot.trn_boot import _ntff_profile_via_ctypes

        mod.set_axon_ntff_profile_hook(
            _ntff_profile_via_ctypes("/opt/axon/libaxon_pjrt.so")
        )
    except Exception:
        pass


def kernel(x: np.ndarray, weight: np.ndarray) -> np.ndarray:
    global LAST_EXEC_NS, LAST_RESULTS, _NC_CACHE
    x = np.ascontiguousarray(np.asarray(x, dtype=np.float32))
    weight = np.ascontiguousarray(np.asarray(weight, dtype=np.float32))
    lead = x.shape[:-1]
    xf = x.reshape(-1, DFULL)
    assert xf.shape[0] == 2 * TOK, xf.shape

    if _NC_CACHE is None:
        _NC_CACHE = _build()
    nc = _NC_CACHE

    in_maps = []
    for i in range(2):
        for j in range(4):
            in_maps.append({
                "x": np.ascontiguousarray(
                    xf[i * TOK:(i + 1) * TOK, j * DIN:(j + 1) * DIN]
                ),
                "weight": np.ascontiguousarray(
                    weight[j * DOUT:(j + 1) * DOUT, :]
                ),
            })
    trace = bool(int(os.environ.get("CCK_TRACE", "0")))
    kw = {}
    if trace:
        _ensure_ntff_hook()
        tdir = os.environ.get("CCK_TRACE_DIR")
        if tdir:
            os.makedirs(tdir, exist_ok=True)
            kw["tmpdir"] = tdir
    res = run_bass_kernel_spmd(nc, in_maps, list(range(N_CORES)), trace=trace, **kw)
    LAST_EXEC_NS = res.exec_time_ns
    LAST_RESULTS = res
    out = np.empty((2 * TOK, DFULL), dtype=np.float32)
    for i in range(2):
        for j in range(4):
            out[i * TOK:(i + 1) * TOK, j * DOUT:(j + 1) * DOUT] = (
                res.results[i * 4 + j]["out"]
            )
    return out.reshape(*lead, DFULL)


if __name__ == "__main__":
    rng = np.random.default_rng(0)
    x = rng.standard_normal((2, 4096, 4096), dtype=np.float32)
    w = (rng.standard_normal((4096, 1024), dtype=np.float32) * 0.02).astype(np.float32)
    o = kernel(x, w)
    print(o.shape, o.dtype, LAST_EXEC_NS)
